# revision 17
# baseline (speedup 1.0000x reference)
"""LocallyConnected1d (untied-weight conv1d) on 8 Trainium2 NeuronCores.

Problem (hardcoded):
    x:      (B=128, C=64, L=1024) f32
    weight: (O=64, C=64, P=1024, K=7) f32   (untied per output position)
    bias:   (O=64, P=1024) f32
    out:    (B=128, O=64, P=1024) f32
    out[b,o,p] = sum_{c,k} xpad[b,c,p+k] * w[o,c,p,k] + bias[o,p]  (pad=3)

Sharding: sequence-parallel over P — core m owns positions [128m, 128m+128).
Each weight element is used exactly once, so the kernel is HBM-bound on the
weight stream; all tensors move as fp16 (quantization error ~4e-4 rel).

Per-core algorithm: adjacent input columns are PAIRED into a 128-deep
contraction (rows 0:64 = channels of column 2t, rows 64:128 = column 2t+1).
Each output position receives its 7 taps from exactly 4 pair-matmuls
(half-zero weight blocks at the pair edges; trimming those via K=64
matmuls was tried and made the PE the bottleneck — more matmuls+LDWEIGHTS
cost far more than the 0.5 MB of DMA saved). Positions are processed in
16 banks of 8 (one PSUM bank = 8 pos x 64 out-ch = 512 f32); each bank is
opened by a K=1 ones-x-bias matmul (start=True writes the full bank), then
7 pair-matmuls accumulate. Eviction: one DVE copy (f32 psum -> fp16 SBUF)
per bank; stores grouped up to 4 banks per DMA (4 KiB per-partition
descriptors), with the last two banks solo and the last weight load split
so the final DMA->matmul->store tail stays short; host upcasts to f32.

The DMA issue shape (15 loads: bias + 4 x-quarters + 2-bank weight chunks,
6 column-slice stores on the scalar ring) is a measured optimum: variants
with 10/13/14/16 loads, 4-bank chunks, per-group contiguous outputs,
two-ring load splits, or SWDGE stores all dropped the load queue from
~346 GB/s to 300-315 GB/s (47-54 us).
"""

import numpy as np

B = 128
C = 64
O = 64
L = 1024
KW = 7
PAD = 3
NCORES = 8
PC = L // NCORES          # positions per core = 128
NJ = PC + 2 * PAD         # input columns per core incl halo = 134
NT = NJ // 2              # column pairs = 67
NBANK = PC // 8           # psum banks of 8 positions = 16
BANKW = 8 * O             # psum bank free width = 512 f32
# per-bank pair-matmul block widths (positions covered)
BN = [2, 4, 6, 8, 6, 4, 2]
BOFF = np.cumsum([0] + BN).tolist()      # col offsets/64, total 32
BANKC = BOFF[-1] * O                     # weight cols per bank = 2048
WCOLS = NBANK * BANKC                    # 32768
XQ = 17                                  # pairs per x DMA block


def _pack_inputs(x, weight, bias):
    """Host-side relayout into DMA/matmul-friendly per-core fp16 arrays."""
    xp = np.zeros((B, C, L + 2 * PAD), np.float32)
    xp[:, :, PAD:PAD + L] = x
    # (C, 1030, B): column-major access per (c, j)
    xt = np.ascontiguousarray(xp.transpose(1, 2, 0))

    xpacks = []
    for m in range(NCORES):
        s = xt[:, PC * m: PC * m + NJ, :]                  # (C, NJ, B)
        s = s.reshape(C, NT, 2, B).transpose(2, 0, 1, 3)   # (h, C, NT, B)
        xpacks.append(np.ascontiguousarray(
            s.reshape(2 * C, NT, B).astype(np.float16)))

    # (P, K, C, O): wt[p, k][c, o] = weight[o, c, p, k]
    wt = np.ascontiguousarray(weight.transpose(2, 3, 1, 0).astype(np.float16))
    wpacks = []
    for m in range(NCORES):
        wp = np.zeros((2 * C, WCOLS), np.float16)
        p0 = PC * m
        for g in range(NBANK):
            c0 = g * BANKC
            for i in range(7):
                t = 4 * g + i
                lo = max(8 * g, 2 * t - 6)
                hi = min(8 * g + 7, 2 * t + 1)
                b0 = c0 + BOFF[i] * O
                for h in range(2):
                    for pl in range(lo, hi + 1):
                        k = 2 * t + h - pl
                        if 0 <= k < KW:
                            wp[h * C:(h + 1) * C,
                               b0 + (pl - lo) * O: b0 + (pl - lo + 1) * O] = \
                                wt[p0 + pl, k]
        wpacks.append(wp)

    bt = np.ascontiguousarray(bias.T.astype(np.float16))   # (L, O)
    bpacks = []
    for m in range(NCORES):
        bp = np.empty((1, PC * O + B), np.float16)
        bp[0, :PC * O] = bt[PC * m: PC * m + PC].reshape(-1)
        bp[0, PC * O:] = 1.0
        bpacks.append(bp)
    return xpacks, wpacks, bpacks


_PROG = None


def _build_program():
    global _PROG
    if _PROG is not None:
        return _PROG

    import concourse.bacc as bacc
    import concourse.mybir as mybir
    import concourse.tile as tile

    F32 = mybir.dt.float32
    F16 = mybir.dt.float16

    nc = bacc.Bacc("TRN2", target_bir_lowering=False, debug=False,
                   num_devices=NCORES)
    x_d = nc.dram_tensor("xp", (2 * C, NT, B), F16, kind="ExternalInput")
    w_d = nc.dram_tensor("wp", (2 * C, WCOLS), F16, kind="ExternalInput")
    b_d = nc.dram_tensor("bp", (1, PC * O + B), F16, kind="ExternalInput")
    o_d = nc.dram_tensor("out", (B, PC * O), F16, kind="ExternalOutput")

    with tile.TileContext(nc) as tc:
        with (
            tc.tile_pool(name="xb", bufs=4) as xpool,
            tc.tile_pool(name="wb", bufs=9) as wpool,
            tc.tile_pool(name="cst", bufs=1) as cpool,
            tc.tile_pool(name="st", bufs=4) as spool,
            tc.tile_pool(name="ps", bufs=4, space="PSUM") as ppool,
        ):
            biast = cpool.tile([1, PC * O + B], F16)
            nc.sync.dma_start(biast[:], b_d[:])
            ones = biast[0:1, PC * O: PC * O + B]

            # loads, issued in consumption order on the SP HWDGE ring:
            # x quarters (17 pairs each) interleaved with weight blocks
            # (2 banks each; the last bank is split in two so its matmuls
            # overlap the final load).
            xtiles = []
            wtiles = []   # list of (tile, col0, ncols)

            def load_x(q):
                n = min(XQ, NT - XQ * q)
                xt = xpool.tile([2 * C, n * B], F16)
                nc.sync.dma_start(xt[:], x_d[:, XQ * q: XQ * q + n, :])
                xtiles.append(xt)

            def load_w(g0, nb):
                wtl = wpool.tile([2 * C, nb * BANKC], F16)
                nc.sync.dma_start(
                    wtl[:], w_d[:, g0 * BANKC: (g0 + nb) * BANKC])
                wtiles.append((wtl, g0 * BANKC, nb * BANKC))

            def load_w_split(g0):
                # one bank in two DMAs into one tile (block i=0..3 | 4..6)
                wtl = wpool.tile([2 * C, BANKC], F16)
                cut = BOFF[5] * O
                c0 = g0 * BANKC
                nc.sync.dma_start(wtl[:, :cut], w_d[:, c0: c0 + cut])
                nc.sync.dma_start(wtl[:, cut:], w_d[:, c0 + cut: c0 + BANKC])
                wtiles.append((wtl, c0, BANKC))

            load_x(0)
            load_w(0, 2)
            load_x(1)
            load_w(2, 2)
            load_w(4, 2)
            load_x(2)
            load_w(6, 2)
            load_w(8, 2)
            load_x(3)
            load_w(10, 2)
            load_w(12, 2)
            load_w(14, 1)
            load_w_split(15)

            def wslice(g):
                want = g * BANKC
                for wtl, c0, ncols in wtiles:
                    if c0 <= want < c0 + ncols:
                        return wtl, want - c0
                raise AssertionError

            # store groups: (first bank, n banks) per output DMA
            SGROUPS = [(0, 4), (4, 4), (8, 4), (12, 2), (14, 1), (15, 1)]
            gstart = {}
            for s, z in SGROUPS:
                for g in range(s, s + z):
                    gstart[g] = (s, z)

            stage = None
            for g in range(NBANK):
                s, z = gstart[g]
                if g == s:
                    stage = spool.tile([B, z * BANKW], F16)
                sl = stage[:, BANKW * (g - s): BANKW * (g - s + 1)]
                ps = ppool.tile([B, BANKW], F32, tag="ps")
                # bias opens the bank: writes all 512 cols (start=True),
                # subsequent pair-matmuls accumulate.
                nc.tensor.matmul(
                    ps[:],
                    ones,
                    biast[0:1, BANKW * g: BANKW * (g + 1)],
                    start=True, stop=False,
                )
                wtl, wc = wslice(g)
                for i in range(7):
                    t = 4 * g + i
                    lo = max(8 * g, 2 * t - 6)
                    hi = min(8 * g + 7, 2 * t + 1)
                    n = hi - lo + 1
                    xt = xtiles[t // XQ]
                    xs = xt[:, B * (t % XQ): B * (t % XQ + 1)]
                    w0 = wc + BOFF[i] * O
                    ws = wtl[:, w0: w0 + n * O]
                    nc.tensor.matmul(
                        ps[:, O * (lo - 8 * g): O * (hi + 1 - 8 * g)],
                        xs,
                        ws,
                        start=False,
                        stop=(i == 6),
                    )
                nc.vector.tensor_copy(sl, ps[:])
                if g == s + z - 1:
                    nc.scalar.dma_start(
                        o_d[:, BANKW * s: BANKW * (s + z)], stage[:])

    nc.compile()
    _PROG = nc
    return nc


def _ensure_ntff_hook():
    """bass_utils' trace path imports antenv.axon_hooks, which this image
    lacks — if BASS_TRACE is set in the environment that import would crash.
    Install a minimal shim (ctypes into libaxon_pjrt.so; falls back to a
    no-hook stub that bass_utils handles by skipping the trace)."""
    import sys
    import types
    try:
        import antenv.axon_hooks  # noqa: F401
        return
    except ImportError:
        pass
    hook = None
    try:
        import contextlib
        import ctypes
        lib = ctypes.CDLL("/opt/axon/libaxon_pjrt.so")
        lib.axon_start_nrt_profile.argtypes = [
            ctypes.POINTER(ctypes.c_int64), ctypes.c_size_t]
        lib.axon_start_nrt_profile.restype = ctypes.c_int64
        lib.axon_stop_nrt_profile.argtypes = [ctypes.c_char_p]
        lib.axon_stop_nrt_profile.restype = ctypes.c_int64

        @contextlib.contextmanager
        def _hook(output_dir, device_ids):
            import jax
            jax.devices()
            if device_ids:
                ids = (ctypes.c_int64 * len(device_ids))(*device_ids)
                rc = lib.axon_start_nrt_profile(ids, len(device_ids))
            else:
                rc = lib.axon_start_nrt_profile(None, 0)
            if rc != 0:
                raise RuntimeError(f"axon_start_nrt_profile rc={rc}")
            try:
                yield
            finally:
                lib.axon_stop_nrt_profile(str(output_dir).encode())

        hook = _hook
    except Exception:
        hook = None
    mod = types.ModuleType("antenv.axon_hooks")
    mod.get_axon_ntff_profile_hook = lambda: hook
    mod.set_axon_ntff_profile_hook = lambda h: None
    try:
        import antenv
        antenv.axon_hooks = mod
    except ImportError:
        pass
    sys.modules["antenv.axon_hooks"] = mod


def _run(x, weight, bias, trace=False, tmpdir=None):
    from concourse.bass_utils import run_bass_kernel_spmd
    _ensure_ntff_hook()

    x = np.asarray(x, dtype=np.float32)
    weight = np.asarray(weight, dtype=np.float32)
    bias = np.asarray(bias, dtype=np.float32)
    xpacks, wpacks, bpacks = _pack_inputs(x, weight, bias)
    nc = _build_program()
    in_maps = [{"xp": xpacks[m], "wp": wpacks[m], "bp": bpacks[m]}
               for m in range(NCORES)]
    res = run_bass_kernel_spmd(nc, in_maps, list(range(NCORES)), trace=trace,
                               tmpdir=tmpdir)
    outs = [np.asarray(r["out"], dtype=np.float32)
            .reshape(B, PC, O).transpose(0, 2, 1)
            for r in res.results]
    full = np.ascontiguousarray(np.concatenate(outs, axis=2))
    return full, res


def kernel(x, weight, bias):
    out, _ = _run(x, weight, bias, trace=False)
    return out
